# revision 19
# baseline (speedup 1.0000x reference)
"""Causal attention kernel for Trainium2 (8 NeuronCores, SPMD over heads).

Problem: B=4, H=16, S=2048, D=64, fp32.
  scores = Q @ K^T / sqrt(64); causal mask; softmax (global-max shift in the
  reference cancels exactly, so plain exp/rowsum is mathematically identical
  and numerically safe: |scores/8| <= ~8); out = attn @ V.

Distribution: B*H = 64 heads -> 8 heads per core, embarrassingly parallel.

Per-core algorithm (per head, four q-passes of 512):
  - Host pre-transposes Q,K to [D,S] per head, so no on-device transposes.
  - scoresT[k,q] = sum_d K[k,d] Q[q,d] via f16 matmuls, k on partitions.
    Contraction is D=64: even k-tiles use PE rows 0-63, odd rows 64-127
    (row packing -> two matmuls run concurrently).
  - k-tiles are processed in groups of 3 sharing one PSUM score tile
    [128,1536] so exp runs as ONE big ACTIVATE per group (amortizes the
    ~290-cycle per-instruction ScalarE overhead).
  - Some groups' exp can be offloaded to the vector engine using the
    Schraudolph exp2 bit trick straight into fp16 bits:
      f16bits = round(s * (log2e/8*1024) + (15*1024 - 62.28))
    (max rel err ~4%, RMS ~2.2%, mean error cancels in softmax).
  - Causal mask: fp16 keep-mask multiply on DVE for diagonal 128x128 blocks.
  - PV: outT[m,q] = sum_k [V|ones]^T(128-contraction) @ at accumulated in
    one PSUM bank; row 64 of outT is the softmax denominator for free.
  - Normalize: copy acc->SBUF f16, DMA-xbar transpose to [q,65] layout,
    reciprocal + per-partition scalar multiply on DVE; DMA out f16.
  - Host converts f16 -> f32 output.
"""

import math
import os
import sys

import numpy as np

if "/opt/trn_rl_repo" not in sys.path:
    sys.path.insert(0, "/opt/trn_rl_repo")

B, H, S, D = 4, 16, 2048, 64
N_CORES = 8
HEADS_PER_CORE = (B * H) // N_CORES  # 8
PASS_Q = 512  # q-columns per pass (1 PSUM bank for the PV accumulator)
GROUP = int(os.environ.get("KB_GROUP", "3"))  # k-tiles per activation group
CHUNK = 512  # matmul moving-operand max (PSUM bank boundary)

LOG2E = 1.4426950408889634
TRICK_C0 = 1024.0 * LOG2E / 8.0  # 184.665
TRICK_C1 = 15.0 * 1024.0 - 62.28  # exponent bias minus Schraudolph shift

# (pass, group) pairs whose exp is computed on the vector engine with the
# exp2 bit trick instead of ScalarE. Empty = exact exp everywhere.
DVE_GROUPS = ()


def _chunks(lo, hi):
    """Split [lo, hi) at absolute multiples of CHUNK (PSUM bank boundaries)."""
    out = []
    c = lo
    while c < hi:
        w = min(hi, (c // CHUNK + 1) * CHUNK) - c
        out.append((c, w))
        c += w
    return out


def build_attention(tc, outs, ins, n_heads=HEADS_PER_CORE, s=S, dve_groups=DVE_GROUPS):
    import concourse.bass as bass
    import concourse.mybir as mybir

    nc = tc.nc
    f32 = mybir.dt.float32
    f16 = mybir.dt.float16
    i16 = mybir.dt.int16
    Exp = mybir.ActivationFunctionType.Exp
    Alu = mybir.AluOpType

    qt_d, kt_d, v_d = ins["qt"], ins["kt"], ins["v"]
    tri_d = ins["ctri"]
    ot_d = outs["ot"]

    n_ktiles = s // 128  # 16
    n_pass = s // PASS_Q  # 4
    dve_groups = set(dve_groups)

    with (
        tc.tile_pool(name="consts", bufs=1) as cpool,
        tc.tile_pool(name="qpool", bufs=2) as qpool,
        tc.tile_pool(name="kpool", bufs=2) as kpool,
        tc.tile_pool(name="vpool", bufs=2) as vpool,
        tc.tile_pool(name="atpool", bufs=5) as atpool,
        tc.tile_pool(name="otrpool", bufs=2) as otrpool,
        tc.tile_pool(name="nrmpool", bufs=2) as nrmpool,
        tc.tile_pool(name="ofpool", bufs=2) as ofpool,
        tc.tile_pool(name="scpool", bufs=2, space="PSUM") as scpool,
        tc.tile_pool(name="accpool", bufs=2, space="PSUM") as accpool,
        tc.tile_pool(name="accBpool", bufs=2, space="PSUM") as accBpool,
    ):
        split_pv = bool(os.environ.get("KB_SPLIT_PV"))
        c_tri = cpool.tile([128, 128], f16, tag="ctri")
        nc.sync.dma_start(c_tri[:], tri_d[:])
        # HAM warm-up: ~4.5us of back-to-back dummy matmuls gets the PE past
        # the cold K=4/8 clock-gate window while the first input DMAs stream
        # in. Results land in a scratch PSUM region that the first real PV
        # overwrites (start=True).
        n_warm = int(os.environ.get("KB_WARM", "40"))
        if n_warm:
            warm = accpool.tile([128, PASS_Q], f32, name="warmup", tag="acc")
            for i in range(n_warm):
                nc.tensor.matmul(
                    warm[:, 0:128], c_tri[:], c_tri[:],
                    start=True, stop=True, skip_group_check=True,
                )
        # Persistent transpose-staging buffers (alternate by pass parity).
        # Rows 65-79 are zeroed once so the xbar transpose never reads
        # uninitialized memory; rows 0-64 are rewritten every pass.
        osbs = []
        if os.environ.get("KB_NO_NORM") != "2":
            for i in range(2):
                t = cpool.tile([80, PASS_Q], f16, tag=f"osb{i}")
                nc.vector.memset(t[64:80, :], 0.0)
                osbs.append(t)

        # deferred PV emission queue: each entry emits the PV matmuls for one
        # k-group (and, for the pass-final group, the normalize/store chain).
        pv_queue = []

        def _pop_pv():
            pv_queue.pop(0)()

        for h in range(n_heads):
            # Q^T duplicated into both partition halves (for row packing).
            qt2 = qpool.tile([128, s], f16)
            nc.gpsimd.dma_start(qt2[0:64, :], qt_d[h])
            nc.gpsimd.dma_start(qt2[64:128, :], qt_d[h])
            # K^T: even k-tiles -> partitions 0-63, odd -> 64-127.
            kt2 = kpool.tile([128, s // 2], f16)
            kt_src = kt_d[h].rearrange("d (t two c) -> d two t c", two=2, c=128)
            kt2_v = kt2.rearrange("p (t c) -> p t c", c=128)
            nc.gpsimd.dma_start(kt2_v[0:64], kt_src[:, 0])
            nc.gpsimd.dma_start(kt2_v[64:128], kt_src[:, 1])
            # V with a ones-column pre-appended on the host: [128, n_ktiles, 65].
            vx = vpool.tile([128, n_ktiles * 65], f16)
            vx_v = vx.rearrange("p (t c) -> p t c", c=65)
            nc.gpsimd.dma_start(vx_v[:], v_d[h].rearrange("(t p) d -> p t d", p=128))

            for p in range(n_pass):
                q0 = p * PASS_Q
                kmax = (p + 1) * (PASS_Q // 128)  # k-tiles 0..kmax-1
                acc = accpool.tile([128, PASS_Q], f32, name=f"acc_{h}_{p}", tag="acc")
                accB = (
                    accBpool.tile([128, PASS_Q], f32, name=f"accB_{h}_{p}", tag="accB")
                    if split_pv
                    else None
                )
                # Group k-tiles sharing one PSUM score tile + one ACTIVATE.
                # Constraint: two k-tiles of opposite parity run CONCURRENTLY
                # on the PE (row packing) and must never write the same PSUM
                # bank. Full-span (512) tiles sit at bank-aligned offsets, so
                # mixed-parity triples are safe. The partial diagonal tiles
                # (spans 384/256/128) are grouped odd-with-odd; the even one
                # goes alone. The odd pair goes LAST so the stop flag k=kmax-1
                # is on the final PV matmul.
                full = list(range(0, 4 * p + 1))
                groups = [full[i : i + GROUP] for i in range(0, len(full), GROUP)]
                groups.append([4 * p + 2])
                groups.append([4 * p + 1, 4 * p + 3])
                for gi, ks in enumerate(groups):
                    spans = {k: q0 + PASS_Q - max(q0, 128 * k) for k in ks}
                    offs = {}
                    fd = 0
                    for k in ks:
                        offs[k] = fd
                        fd += spans[k]
                    sc = scpool.tile(
                        [128, GROUP * CHUNK], f32, name=f"sc_{h}_{p}_{gi}", tag="sc"
                    )
                    at = atpool.tile([128, GROUP * CHUNK], f16)

                    # QK matmuls, round-robin over the group's k-tiles so that
                    # adjacent instructions target opposite PE row halves.
                    chunk_lists = {k: _chunks(offs[k], offs[k] + spans[k]) for k in ks}
                    n_ch = max(len(v) for v in chunk_lists.values())
                    for ci in range(n_ch):
                        for k in ks:
                            if ci >= len(chunk_lists[k]):
                                continue
                            c, w = chunk_lists[k][ci]
                            half = k % 2
                            qlo = max(q0, 128 * k)
                            qa = qlo + (c - offs[k])
                            nc.tensor.matmul(
                                sc[:, c : c + w],
                                kt2_v[64 * half : 64 * half + 64, k // 2],
                                qt2[64 * half : 64 * half + 64, qa : qa + w],
                                start=True,
                                stop=True,
                                skip_group_check=True,
                            )

                    # exp (exact on ScalarE, or exp2 bit trick on DVE)
                    if (p, gi) in dve_groups:
                        nc.vector.tensor_scalar(
                            at[:, 0:fd].bitcast(i16),
                            sc[:, 0:fd],
                            TRICK_C0,
                            TRICK_C1,
                            Alu.mult,
                            Alu.add,
                        )
                    else:
                        nc.scalar.activation(
                            at[:, 0:fd], sc[:, 0:fd], Exp, scale=0.125
                        )

                    # causal keep-mask for diagonal 128x128 blocks
                    for k in ks:
                        if 128 * k >= q0:
                            nc.vector.tensor_mul(
                                at[:, offs[k] : offs[k] + 128],
                                at[:, offs[k] : offs[k] + 128],
                                c_tri[:],
                            )

                    last_in_pass = gi == len(groups) - 1

                    def _emit(
                        ks=ks, offs=offs, spans=spans, at=at, acc=acc, accB=accB,
                        vx_v=vx_v, h=h, p=p, q0=q0, kmax=kmax, last=last_in_pass,
                    ):
                        for k in ks:
                            qlo = max(q0, 128 * k)
                            if split_pv:
                                nc.tensor.matmul(
                                    acc[0:65, qlo - q0 : PASS_Q],
                                    vx_v[0:64, k, :],
                                    at[0:64, offs[k] : offs[k] + spans[k]],
                                    start=(k == 0),
                                    stop=(k == kmax - 1),
                                    skip_group_check=True,
                                )
                                nc.tensor.matmul(
                                    accB[0:65, qlo - q0 : PASS_Q],
                                    vx_v[64:128, k, :],
                                    at[64:128, offs[k] : offs[k] + spans[k]],
                                    start=(k == 0),
                                    stop=(k == kmax - 1),
                                    skip_group_check=True,
                                )
                            else:
                                nc.tensor.matmul(
                                    acc[0:65, qlo - q0 : PASS_Q],
                                    vx_v[:, k, :],
                                    at[:, offs[k] : offs[k] + spans[k]],
                                    start=(k == 0),
                                    stop=(k == kmax - 1),
                                    skip_group_check=True,
                                )
                        if not last:
                            return
                        # normalize + store for this pass
                        if os.environ.get("KB_NO_NORM") == "2":
                            nc.sync.dma_start(
                                ot_d[h, q0 : q0 + PASS_Q].rearrange(
                                    "(t p) d -> p t d", p=64
                                ),
                                at[0:64, 0:PASS_Q].rearrange("p (t c) -> p t c", c=64),
                            )
                            return
                        osb = osbs[(h * n_pass + p) % 2]
                        nc.vector.tensor_copy(osb[0:65, :], acc[0:65, :])
                        if os.environ.get("KB_NO_NORM"):
                            nc.sync.dma_start(
                                ot_d[h, q0 : q0 + PASS_Q].rearrange(
                                    "(t p) d -> p t d", p=64
                                ),
                                osb[0:64, :].rearrange("p (t c) -> p t c", c=64),
                            )
                            return
                        otr = otrpool.tile([128, 4 * 80], f16, tag="otr",
                                           name=f"otr_{h}_{p}")
                        otr_v = otr.rearrange("p (t c) -> p t c", c=80)
                        nc.sync.dma_start_transpose(otr_v[:], osb[:, :])
                        rcl = nrmpool.tile([128, 4], f32, tag="rcl",
                                           name=f"rcl_{h}_{p}")
                        nc.vector.tensor_copy(rcl[:], otr_v[:, :, 64:65])
                        rcp = nrmpool.tile([128, 4], f32, tag="rcp",
                                           name=f"rcp_{h}_{p}")
                        rsc = nrmpool.tile([128, 4], f32, tag="rsc",
                                           name=f"rsc_{h}_{p}")
                        nc.vector.reciprocal_approx_accurate(rcp[:], rcl[:], rsc[:])
                        of = ofpool.tile([128, 4 * 64], f16, tag="of",
                                         name=f"of_{h}_{p}")
                        of_v = of.rearrange("p (t c) -> p t c", c=64)
                        for j in range(4):
                            nc.vector.tensor_scalar(
                                of_v[:, j],
                                otr_v[:, j, 0:64],
                                rcp[:, j : j + 1],
                                None,
                                Alu.mult,
                            )
                        nc.gpsimd.dma_start(
                            ot_d[h, q0 : q0 + PASS_Q].rearrange(
                                "(t p) d -> p t d", p=128
                            ),
                            of_v[:],
                        )

                    pv_queue.append(_emit)
                    if len(pv_queue) > 2:
                        _pop_pv()
        while pv_queue:
            _pop_pv()


def _make_consts():
    kk, qq = np.meshgrid(np.arange(128), np.arange(128), indexing="ij")
    tri = (kk <= qq).astype(np.float16)  # keep-mask for the diagonal block
    return tri


_NC_CACHE = {}


def _build_nc(n_heads=HEADS_PER_CORE, s=S, dve_groups=DVE_GROUPS):
    key = (n_heads, s, tuple(sorted(dve_groups)))
    if key in _NC_CACHE:
        return _NC_CACHE[key]
    import concourse.tile as tile
    from concourse import bacc, mybir

    nc = bacc.Bacc(
        "TRN2", target_bir_lowering=False, debug=False, enable_asserts=False
    )
    f32 = mybir.dt.float32
    f16 = mybir.dt.float16
    ins = {
        "qt": nc.dram_tensor("qt", [n_heads, D, s], f16, kind="ExternalInput").ap(),
        "kt": nc.dram_tensor("kt", [n_heads, D, s], f16, kind="ExternalInput").ap(),
        "v": nc.dram_tensor("v", [n_heads, s, D + 1], f16, kind="ExternalInput").ap(),
        "ctri": nc.dram_tensor("ctri", [128, 128], f16, kind="ExternalInput").ap(),
    }
    outs = {
        "ot": nc.dram_tensor("ot", [n_heads, s, D], f16, kind="ExternalOutput").ap(),
    }
    with tile.TileContext(nc) as tc:
        build_attention(tc, outs, ins, n_heads=n_heads, s=s, dve_groups=dve_groups)
    nc.compile()
    _NC_CACHE[key] = nc
    return nc


def kernel(Q, K, V, mask, trace=False):
    """Full-input entry point: shards over 8 NeuronCores, returns full output."""
    from concourse.bass_utils import run_bass_kernel_spmd

    nc = _build_nc()
    tri = _make_consts()

    Qf = np.ascontiguousarray(
        Q.reshape(B * H, S, D).transpose(0, 2, 1), dtype=np.float16
    )
    Kf = np.ascontiguousarray(
        K.reshape(B * H, S, D).transpose(0, 2, 1), dtype=np.float16
    )
    Vf = np.concatenate(
        [
            V.reshape(B * H, S, D).astype(np.float16),
            np.ones((B * H, S, 1), dtype=np.float16),
        ],
        axis=-1,
    )

    in_maps = []
    for c in range(N_CORES):
        sl = slice(c * HEADS_PER_CORE, (c + 1) * HEADS_PER_CORE)
        in_maps.append(
            {
                "qt": Qf[sl],
                "kt": Kf[sl],
                "v": Vf[sl],
                "ctri": tri,
            }
        )

    res = run_bass_kernel_spmd(nc, in_maps, core_ids=list(range(N_CORES)), trace=trace)
    ot = np.concatenate([res.results[c]["ot"] for c in range(N_CORES)], axis=0)
    out = ot.reshape(B, H, S, D)
    kernel.last_results = res
    return np.ascontiguousarray(out, dtype=np.float32)
